# revision 13
# baseline (speedup 1.0000x reference)
"""Trainium2 Bass kernel for the CMLIF (masked LIF over conv-mask) module.

Math being implemented:
    mask = (sigmoid(conv2d(ones) + b) > 0.5)            # batch-independent
    u_0 = 0
    u_{t+1} = 0.5 * u_t * (u_t <= 1) + x_t              # leaky integrate+reset
    o_t = (u_{t+1} > 1) * mask

Device trick: substitute q_t = 2^t * u_t (power-of-2 scaling is exact in
fp32).  Then
    q_{t+1} = q_t * (q_t <= 2^t) + 2^{t+1} * x_t
    o_t     = (q_{t+1} > 2^{t+1}) * mask
The leak multiplier disappears: the reset+leak is one fused DVE
scalar_tensor_tensor (q <= thr) * q, followed by one tensor_tensor add of
the host-prescaled x (exact).  This is bit-exact vs the reference.

The output compare runs on GPSIMD as tensor_scalar against a per-channel
threshold (2^(t+1) where the mask interior is 1, +huge where 0) -- the
conv-of-ones mask is channel-constant except at image borders.  DVE then
patches the border rows/cols (2 strided fused ops per step) using the true
border mask encoded as thresholds.  Output is written as int8 (exactly
0/1) to cut HBM write traffic 4x; the host upcasts to f32.

Sharding: data-parallel over batch across 8 NeuronCores; each core runs
the full 5-step recurrence on bs/8 images.  No cross-core communication.
"""

import numpy as np

TIME_STEP = 5
N_CORES = 8

LAST_RESULTS = None

_NC_CACHE = {}


def _import_concourse():
    try:
        import concourse.bass  # noqa: F401
    except ImportError:
        import sys

        for p in ("/opt/trn_rl_repo", "/root/.axon_site/_ro/trn_rl_repo"):
            if p not in sys.path:
                sys.path.append(p)
    import concourse.bacc as bacc
    import concourse.mybir as mybir
    from concourse.tile import TileContext
    from concourse.bass_utils import run_bass_kernel_spmd

    return bacc, mybir, TileContext, run_bass_kernel_spmd


def build_nc(
    B_l,
    C,
    HW,
    G,
    H,
    u_bufs=2,
    x_bufs=2,
    o_bufs=3,
    repeat=1,
):
    """Build the per-core Bass program.

    DRAM layout (per core):
      x     [T, C, B_l*HW]  f32  -- t-major, channel on partitions, local
                                    batch folded into the free axis;
                                    host-prescaled by 2^(t+1).
      nthv  [C, T]          f32  -- negated per-channel interior threshold:
                                    -2^(t+1) if interior-mask else -1e33.
      thbr  [C, G*2*Wd]     f32  -- border-row thresholds (1 or 1e30),
                                    layout [g][h in {0, H-1}][w].
      thbc  [C, G*H*2]      f32  -- border-col thresholds (1 or 1e30),
                                    layout [g][h][w in {0, Wd-1}].
      o     [T, C, B_l*HW]  int8 -- encoded spikes: interior Sign(q-th) in
                                    {-1,0,1}, borders {0,1}; host decodes
                                    spike = (o == 1).

    Engine split: DVE runs the recurrence (fused reset STT + add) plus the
    tiny border patches; ACT (ScalarE) computes the interior spike via
    Sign(q + bias_c) with the per-channel bias; SP issues all DMA.
    GPSIMD is unused -- measured ~20x below line rate for bulk
    tensor_scalar on this stack.
    """
    bacc, mybir, TileContext, _ = _import_concourse()
    f32, i8 = mybir.dt.float32, mybir.dt.int8
    Alu = mybir.AluOpType
    T = TIME_STEP
    W = G * HW
    Wd = HW // H
    nG = B_l // G
    assert B_l % G == 0

    nc = bacc.Bacc()
    xs = nc.declare_dram_parameter("x", [T, C, B_l * HW], f32, isOutput=False)
    ntv = nc.declare_dram_parameter("nthv", [C, T], f32, isOutput=False)
    tbr = nc.declare_dram_parameter("thbr", [C, G * 2 * Wd], f32, isOutput=False)
    tbc = nc.declare_dram_parameter("thbc", [C, G * H * 2], f32, isOutput=False)
    oo = nc.declare_dram_parameter("o", [T, C, B_l * HW], i8, isOutput=True)

    with TileContext(nc) as tc:
        with (
            tc.tile_pool(name="const", bufs=1) as cpool,
            tc.tile_pool(name="u", bufs=u_bufs) as upool,
            tc.tile_pool(name="xt", bufs=x_bufs) as xpool,
            tc.tile_pool(name="ot", bufs=o_bufs) as opool,
        ):
            nthv_t = cpool.tile([C, T], f32)
            nc.sync.dma_start(out=nthv_t[:], in_=ntv[:])
            thbr_t = cpool.tile([C, G * 2 * Wd], f32)
            nc.sync.dma_start(out=thbr_t[:], in_=tbr[:])
            thbc_t = cpool.tile([C, G * H * 2], f32)
            nc.sync.dma_start(out=thbc_t[:], in_=tbc[:])
            thbr_ap = thbr_t[:].rearrange("c (g r w) -> c g r w", g=G, r=2)
            thbc_ap = thbc_t[:].rearrange("c (g h r) -> c g h r", g=G, h=H)

            for g in [g for _ in range(repeat) for g in range(nG)]:
                sl = slice(g * W, (g + 1) * W)
                u = upool.tile([C, W], f32, tag="u")
                # q_1 = 2*x_0 (host already scaled x_0 by 2)
                nc.sync.dma_start(out=u[:], in_=xs[0, :, sl])
                u4 = u[:].rearrange("c (g h w) -> c g h w", g=G, h=H)
                for t in range(T):
                    sc = float(2 ** (t + 1))
                    if t > 0:
                        xt = xpool.tile([C, W], f32, tag="xt")
                        nc.sync.dma_start(out=xt[:], in_=xs[t, :, sl])
                        # q~ = (q <= 2^t) * q
                        nc.vector.scalar_tensor_tensor(
                            u[:], u[:], float(2**t), u[:], Alu.is_le, Alu.mult
                        )
                        # q += 2^(t+1) * x_t
                        nc.vector.tensor_tensor(u[:], u[:], xt[:], Alu.add)
                    ot = opool.tile([C, W], i8, tag="ot")
                    # interior spike on ACT: Sign(q - th_c) in {-1,0,1}
                    nc.scalar.sign(ot[:], u[:], nthv_t[:, t : t + 1])
                    # DVE patches borders with the true mask:
                    # o = (thb * 2^(t+1)) < q  in {0,1}
                    # (STT requires <=3D APs, so per-image ops)
                    ot4 = ot[:].rearrange("c (g h w) -> c g h w", g=G, h=H)
                    for g2 in range(G):
                        nc.vector.scalar_tensor_tensor(
                            ot4[:, g2, 0 :: H - 1, :],
                            thbr_ap[:, g2],
                            sc,
                            u4[:, g2, 0 :: H - 1, :],
                            Alu.mult,
                            Alu.is_lt,
                        )
                        nc.vector.scalar_tensor_tensor(
                            ot4[:, g2, :, 0 :: Wd - 1],
                            thbc_ap[:, g2],
                            sc,
                            u4[:, g2, :, 0 :: Wd - 1],
                            Alu.mult,
                            Alu.is_lt,
                        )
                    nc.sync.dma_start(out=oo[t, :, sl], in_=ot[:])
    nc.compile()
    return nc


def compute_mask(conv_w, conv_b, H, W):
    """mask[c,h,w] = sigmoid(conv2d(ones)+b)[c,h,w] > 0.5  ==  z > 0.

    conv(ones) only depends on how much of the 3x3 kernel window is in
    bounds, so z = sum over valid (kh,kw) of s[c,kh,kw] + b[c], with
    s = conv_w.sum(axis=1).  Computed in f64 for a stable sign.
    """
    C = conv_w.shape[0]
    s = conv_w.astype(np.float64).sum(axis=1)  # [C,3,3]
    VH = np.zeros((H, 3))
    VW = np.zeros((W, 3))
    for k in range(3):
        VH[max(0, 1 - k) : min(H, H + 1 - k), k] = 1.0
        VW[max(0, 1 - k) : min(W, W + 1 - k), k] = 1.0
    z = np.einsum("ckl,hk,wl->chw", s, VH, VW) + conv_b.astype(np.float64)[:, None, None]
    return (z > 0).astype(np.float32).reshape(C, H * W)


def mask_aux(mask2d, H, Wd, G):
    """Threshold encodings of the mask.

    nthv [C,T]: -2^(t+1) where interior mask is 1 else -1e33 (ACT bias).
    thbr [C, G*2*Wd]: border rows (h=0, h=H-1), 1.0 where mask else 1e30.
    thbc [C, G*H*2]:  border cols (w=0, w=Wd-1), same encoding.
    """
    C = mask2d.shape[0]
    m3 = mask2d.reshape(C, H, Wd)
    interior = m3[:, H // 2, Wd // 2]
    scales = (2.0 ** np.arange(1, TIME_STEP + 1)).astype(np.float32)
    nthv = np.where(
        interior[:, None] > 0, -scales[None, :], np.float32(-1e33)
    ).astype(np.float32)
    th3 = np.where(m3 > 0, np.float32(1.0), np.float32(1e30))
    rows = th3[:, [0, H - 1], :]  # [C, 2, Wd]
    cols = th3[:, :, [0, Wd - 1]]  # [C, H, 2]
    thbr = np.tile(rows.reshape(C, -1), (1, G)).astype(np.float32)
    thbc = np.tile(cols.reshape(C, -1), (1, G)).astype(np.float32)
    return nthv, thbr, thbc


def kernel(x, conv_w, conv_b):
    global LAST_RESULTS
    _, _, _, run_bass_kernel_spmd = _import_concourse()

    T = TIME_STEP
    n, C, H, Wd = x.shape
    bs = n // T
    HW = H * Wd
    assert bs % N_CORES == 0, (bs, N_CORES)
    B_l = bs // N_CORES
    G = 2 if B_l % 2 == 0 else 1

    mask2d = compute_mask(conv_w, conv_b, H, Wd)
    nthv, thbr, thbc = mask_aux(mask2d, H, Wd, G)

    key = (B_l, C, HW, G, H)
    if key not in _NC_CACHE:
        _NC_CACHE[key] = build_nc(*key)
    nc = _NC_CACHE[key]

    # 2^(t+1) scaling, exact in fp32
    scales = (2.0 ** np.arange(1, T + 1)).astype(np.float32)
    x5 = x.reshape(T, bs, C, HW)
    in_maps = []
    for k in range(N_CORES):
        b0 = k * B_l
        # [T, C, B_l, HW], scaled; ufunc output is C-contiguous
        xc = x5[:, b0 : b0 + B_l].transpose(0, 2, 1, 3) * scales[:, None, None, None]
        in_maps.append(
            {
                "x": xc.reshape(T, C, B_l * HW),
                "nthv": nthv,
                "thbr": thbr,
                "thbc": thbc,
            }
        )

    res = run_bass_kernel_spmd(nc, in_maps, list(range(N_CORES)))
    LAST_RESULTS = res

    # decode: interior carries Sign(q-th) in {-1,0,1}; spike = (code == 1)
    out = np.empty((T, bs, C, HW), np.float32)
    for k in range(N_CORES):
        b0 = k * B_l
        ok = (res.results[k]["o"].reshape(T, C, B_l, HW) == 1).transpose(0, 2, 1, 3)
        out[:, b0 : b0 + B_l] = ok
    return out.reshape(n, C, H, Wd)


# revision 19
# speedup vs baseline: 265.1341x; 265.1341x over previous
"""Trainium2 Bass kernel for the CMLIF (masked LIF over conv-mask) module.

Math being implemented:
    mask = (sigmoid(conv2d(ones) + b) > 0.5)            # batch-independent
    u_0 = 0
    u_{t+1} = 0.5 * u_t * (u_t <= 1) + x_t              # leaky integrate+reset
    o_t = (u_{t+1} > 1) * mask

Device trick: substitute q_t = 2^t * u_t (power-of-2 scaling is exact in
fp32).  Then
    q_{t+1} = q_t * (q_t <= 2^t) + 2^{t+1} * x_t
    o_t     = (q_{t+1} > 2^{t+1}) * mask
The leak multiplier disappears: the reset+leak is one fused DVE
scalar_tensor_tensor (q <= thr) * q, followed by one tensor_tensor add of
the host-prescaled x (exact).  This is bit-exact vs the reference.

The output compare runs on the otherwise-idle ScalarE (ACT) as
Sign(q + bias_c) with a per-channel bias (-2^(t+1) where the mask
interior is 1, -huge where 0) -- the conv-of-ones mask is
channel-constant except at image borders.  DVE then patches the border
rows/cols (2 strided fused STTs per step) using the true border mask
encoded as thresholds.  Output is written as int8 (interior codes
{-1,0,1}, borders {0,1}) to cut HBM write traffic 4x; the host decodes
spike = (code == 1).

Sharding: data-parallel over batch across 8 NeuronCores; each core runs
the full 5-step recurrence on bs/8 images.  No cross-core communication.
"""

import numpy as np

TIME_STEP = 5
N_CORES = 8

LAST_RESULTS = None

_NC_CACHE = {}


def _import_concourse():
    try:
        import concourse.bass  # noqa: F401
    except ImportError:
        import sys

        for p in ("/opt/trn_rl_repo", "/root/.axon_site/_ro/trn_rl_repo"):
            if p not in sys.path:
                sys.path.append(p)
    import concourse.bacc as bacc
    import concourse.mybir as mybir
    from concourse.tile import TileContext
    from concourse.bass_utils import run_bass_kernel_spmd

    return bacc, mybir, TileContext, run_bass_kernel_spmd


def build_nc(
    B_l,
    C,
    HW,
    G,
    H,
    u_bufs=4,
    x_bufs=2,
    o_bufs=3,
    repeat=1,
):
    """Build the per-core Bass program.  (G is fixed at 1 in this layout.)

    DRAM layout (per core; image-major so timesteps are contiguous):
      x     [B_l, T, C, HW]  f32  -- host-prescaled by 2^(t+1); per image
                                     the 5 step-frames are contiguous, so
                                     steps (1,2) and (3,4) load as single
                                     4 MB DMAs.
      nthv  [C, T]           f32  -- negated per-channel interior threshold:
                                     -2^(t+1) if interior-mask else -1e33.
      thbr  [C, 2*Wd]        f32  -- border-row thresholds (1 or 1e30),
                                     layout [h in {0, H-1}][w].
      thbc  [C, H*2]         f32  -- border-col thresholds (1 or 1e30),
                                     layout [h][w in {0, Wd-1}].
      o     [B_l, T, C, HW]  int8 -- encoded spikes: interior Sign(q-th) in
                                     {-1,0,1}, borders {0,1}; host decodes
                                     spike = (o == 1).  All 5 steps of an
                                     image store as one 2.5 MB DMA.

    Engine split: DVE runs the recurrence (fused reset STT + add) plus the
    tiny border patches; ACT (ScalarE) computes the interior spike via
    Sign(q + bias_c) with the per-channel bias and issues the output
    stores on its HWDGE ring; SP issues the loads on its ring.  GPSIMD is
    unused -- measured ~20x below line rate for bulk tensor_scalar on
    this stack.
    """
    bacc, mybir, TileContext, _ = _import_concourse()
    f32, i8 = mybir.dt.float32, mybir.dt.int8
    Alu = mybir.AluOpType
    T = TIME_STEP
    assert G == 1
    W = HW
    Wd = HW // H

    nc = bacc.Bacc()
    xs = nc.declare_dram_parameter("x", [B_l, T, C, HW], f32, isOutput=False)
    ntv = nc.declare_dram_parameter("nthv", [C, T], f32, isOutput=False)
    tbr = nc.declare_dram_parameter("thbr", [C, 2 * Wd], f32, isOutput=False)
    tbc = nc.declare_dram_parameter("thbc", [C, H * 2], f32, isOutput=False)
    oo = nc.declare_dram_parameter("o", [B_l, T, C, HW], i8, isOutput=True)

    with TileContext(nc) as tc:
        with (
            tc.tile_pool(name="const", bufs=1) as cpool,
            tc.tile_pool(name="u", bufs=u_bufs) as upool,
            tc.tile_pool(name="xt", bufs=x_bufs) as xpool,
            tc.tile_pool(name="ot", bufs=o_bufs) as opool,
        ):
            nthv_t = cpool.tile([C, T], f32)
            nc.sync.dma_start(out=nthv_t[:], in_=ntv[:])
            thbr_t = cpool.tile([C, 2 * Wd], f32)
            nc.sync.dma_start(out=thbr_t[:], in_=tbr[:])
            thbc_t = cpool.tile([C, H * 2], f32)
            nc.sync.dma_start(out=thbc_t[:], in_=tbc[:])
            thbr_ap = thbr_t[:].rearrange("c (r w) -> c r w", r=2)
            thbc_ap = thbc_t[:].rearrange("c (h r) -> c h r", h=H)

            for g in [g for _ in range(repeat) for g in range(B_l)]:
                u = upool.tile([C, W], f32, tag="u")
                # q_1 = 2*x_0 (host already scaled x_0 by 2)
                nc.sync.dma_start(out=u[:], in_=xs[g, 0])
                u3 = u[:].rearrange("c (h w) -> c h w", h=H)
                osx = opool.tile([C, T * W], i8, tag="osx")
                for t in range(T):
                    sc = float(2 ** (t + 1))
                    if t > 0:
                        if t % 2 == 1:
                            # steps (1,2) / (3,4) arrive as one 4 MB DMA
                            xp = xpool.tile([C, 2 * W], f32, tag="xp")
                            nc.sync.dma_start(
                                out=xp[:].rearrange("c (t f) -> c t f", t=2),
                                in_=xs[g, t : t + 2].rearrange("t c f -> c t f"),
                            )
                        xt = xp[:, ((t - 1) % 2) * W : ((t - 1) % 2 + 1) * W]
                        # q~ = (q <= 2^t) * q
                        nc.vector.scalar_tensor_tensor(
                            u[:], u[:], float(2**t), u[:], Alu.is_le, Alu.mult
                        )
                        # q += 2^(t+1) * x_t
                        nc.vector.tensor_tensor(u[:], u[:], xt, Alu.add)
                    ot = osx[:, t * W : (t + 1) * W]
                    # interior spike on ACT: Sign(q - th_c) in {-1,0,1}
                    nc.scalar.sign(ot, u[:], nthv_t[:, t : t + 1])
                    # DVE patches borders with the true mask:
                    # o = (thb * 2^(t+1)) < q  in {0,1}
                    ot3 = ot.rearrange("c (h w) -> c h w", h=H)
                    nc.vector.scalar_tensor_tensor(
                        ot3[:, 0 :: H - 1, :],
                        thbr_ap,
                        sc,
                        u3[:, 0 :: H - 1, :],
                        Alu.mult,
                        Alu.is_lt,
                    )
                    nc.vector.scalar_tensor_tensor(
                        ot3[:, :, 0 :: Wd - 1],
                        thbc_ap,
                        sc,
                        u3[:, :, 0 :: Wd - 1],
                        Alu.mult,
                        Alu.is_lt,
                    )
                # one 2.5 MB store for the whole image, on the ACT ring
                nc.scalar.dma_start(
                    out=oo[g].rearrange("t c f -> c t f"),
                    in_=osx[:].rearrange("c (t f) -> c t f", t=TIME_STEP),
                )
    nc.compile()
    return nc


def compute_mask(conv_w, conv_b, H, W):
    """mask[c,h,w] = sigmoid(conv2d(ones)+b)[c,h,w] > 0.5  ==  z > 0.

    conv(ones) only depends on how much of the 3x3 kernel window is in
    bounds, so z = sum over valid (kh,kw) of s[c,kh,kw] + b[c], with
    s = conv_w.sum(axis=1).  Computed in f64 for a stable sign.
    """
    C = conv_w.shape[0]
    s = conv_w.astype(np.float64).sum(axis=1)  # [C,3,3]
    VH = np.zeros((H, 3))
    VW = np.zeros((W, 3))
    for k in range(3):
        VH[max(0, 1 - k) : min(H, H + 1 - k), k] = 1.0
        VW[max(0, 1 - k) : min(W, W + 1 - k), k] = 1.0
    z = np.einsum("ckl,hk,wl->chw", s, VH, VW) + conv_b.astype(np.float64)[:, None, None]
    return (z > 0).astype(np.float32).reshape(C, H * W)


def mask_aux(mask2d, H, Wd):
    """Threshold encodings of the mask.

    nthv [C,T]: -2^(t+1) where interior mask is 1 else -1e33 (ACT bias).
    thbr [C, 2*Wd]: border rows (h=0, h=H-1), 1.0 where mask else 1e30.
    thbc [C, H*2]:  border cols (w=0, w=Wd-1), same encoding.
    """
    C = mask2d.shape[0]
    m3 = mask2d.reshape(C, H, Wd)
    interior = m3[:, H // 2, Wd // 2]
    scales = (2.0 ** np.arange(1, TIME_STEP + 1)).astype(np.float32)
    nthv = np.where(
        interior[:, None] > 0, -scales[None, :], np.float32(-1e33)
    ).astype(np.float32)
    th3 = np.where(m3 > 0, np.float32(1.0), np.float32(1e30))
    rows = th3[:, [0, H - 1], :]  # [C, 2, Wd]
    cols = th3[:, :, [0, Wd - 1]]  # [C, H, 2]
    thbr = np.ascontiguousarray(rows.reshape(C, -1)).astype(np.float32)
    thbc = np.ascontiguousarray(cols.reshape(C, -1)).astype(np.float32)
    return nthv, thbr, thbc


def make_in_maps(x, conv_w, conv_b):
    """Per-core input dicts in the device layout, plus geometry."""
    T = TIME_STEP
    n, C, H, Wd = x.shape
    bs = n // T
    HW = H * Wd
    assert bs % N_CORES == 0, (bs, N_CORES)
    B_l = bs // N_CORES

    mask2d = compute_mask(conv_w, conv_b, H, Wd)
    nthv, thbr, thbc = mask_aux(mask2d, H, Wd)

    # 2^(t+1) scaling, exact in fp32
    scales = (2.0 ** np.arange(1, T + 1)).astype(np.float32)
    x5 = x.reshape(T, bs, C, HW)
    in_maps = []
    for k in range(N_CORES):
        b0 = k * B_l
        # [B_l, T, C, HW] image-major, scaled; ufunc output is C-contiguous
        xc = x5[:, b0 : b0 + B_l].transpose(1, 0, 2, 3) * scales[None, :, None, None]
        in_maps.append({"x": xc, "nthv": nthv, "thbr": thbr, "thbc": thbc})
    return in_maps, (B_l, C, HW, H, bs)


def kernel(x, conv_w, conv_b):
    global LAST_RESULTS
    _, _, _, run_bass_kernel_spmd = _import_concourse()

    T = TIME_STEP
    n, C, H, Wd = x.shape
    HW = H * Wd

    in_maps, (B_l, C, HW, H, bs) = make_in_maps(x, conv_w, conv_b)

    key = (B_l, C, HW, 1, H)
    if key not in _NC_CACHE:
        _NC_CACHE[key] = build_nc(*key)
    nc = _NC_CACHE[key]

    res = run_bass_kernel_spmd(nc, in_maps, list(range(N_CORES)))
    LAST_RESULTS = res

    # decode: interior carries Sign(q-th) in {-1,0,1}; spike = (code == 1)
    out = np.empty((T, bs, C, HW), np.float32)
    for k in range(N_CORES):
        b0 = k * B_l
        ok = (res.results[k]["o"] == 1).transpose(1, 0, 2, 3)  # [T,B_l,C,HW]
        out[:, b0 : b0 + B_l] = ok
    return out.reshape(n, C, H, Wd)
